# revision 7
# baseline (speedup 1.0000x reference)
"""VQ codebook (DKVB) kernel for Trainium2, sharded over 8 NeuronCores.

Problem: embeddings [8, 2048, 4, 4] -> tokens x [128, 256, 8]; per codebook c
(256 of them), find nearest code among 4096 (euclidean), gather values row.

Strategy: shard the 256 codebooks across 8 cores (32 per core).
Score s[t,k] = x_t . k - |k|^2/2 (argmax s == argmin dist).

Precision: the PE multiplies bf16 operands at fp22 (14-bit significands),
so a plain bf16 Dekker split leaves ~3e-4 score noise (≈100 wrong argmins).
Instead each operand is split 3 ways at 6 mantissa bits:
    x = x0 + x1 + x2,  k = k0 + k1 + k2
(x0, x1 6-bit significands, x2 the exact bf16 remainder; the split is exact
since 7+7+9 significand bits ≥ fp32's 24). All retained products (x0k0,
x0k1, x1k0, x0k2, x2k0, x1k1) have ≤14-bit-significand factors pairs whose
products are fp22-exact or negligible, giving fp32-grade scores. Folded into
ONE bf16 matmul of contraction dim 51:
    lhsT rows = [x0(8); 1; x0(8); 1; x1(8); 1; x0(8); x2(8); x1(8)]
    rhs  rows = [k0(8); -s0; k1(8); -s1; k0(8); -s2; k2(8); k0(8); k1(8)]
with s = |k|^2/2 split into three bf16 parts s0+s1+s2.

Argmax over 4096 codes in ONE VectorE pass with a custom DVE op:
    body = select(Src0 >= scan(MAX, Src0), Idx, -FLT_MAX), accum = MAX
(the last position that establishes a new running max == the argmax).
PE fills PSUM in 2-bank quarters; ScalarE drains each quarter to SBUF as
soon as it completes so the VectorE scan starts with minimal latency.
Values gathered via per-codebook indirect DMA on GpSimd (which also does
the fp32->u32 index cast, keeping VectorE dedicated to the argmax scans).
"""

import numpy as np
import ml_dtypes

import concourse.bass as bass
import concourse.tile as tile
from concourse import bacc, mybir
from concourse.bass_utils import run_bass_kernel_spmd

B, D, H, W = 8, 2048, 4, 4
C, K, d = 256, 4096, 8
NCORES = 8
CBC = C // NCORES          # 32 codebooks per core
T = B * H * W              # 128 tokens
KA = 51                    # 3-way-split-augmented contraction dim
KA_SPLIT = 27              # kT DMA split row (sync ring vs gpsimd ring)
NQ = 4                     # PSUM quarters per codebook
QW = K // NQ               # 1024 columns per quarter

F32 = mybir.dt.float32
BF16 = mybir.dt.bfloat16
U32 = mybir.dt.uint32

_FLT_MAX = np.float32(3.4028235e38)


def _register_argmax_op():
    """Register the single-pass argmax custom DVE op (idempotent)."""
    from concourse import dve_ops
    from concourse.dve_spec import (
        Spec, Src0, MaxNeg, Idx, AluOp, select, lower, maxx, scan,
    )
    from concourse.dve_uop import DveOpSpec

    for op in dve_ops.OPS:
        if op.name == "ARGMAX_SCAN_ANT":
            return op

    def _ref(in0, in1, s0, s1, imm2):
        s = in0.astype(np.float32)
        r = np.maximum.accumulate(s, axis=-1)
        idx = np.broadcast_to(
            np.arange(s.shape[-1], dtype=np.float32), s.shape
        )
        body = np.where(s >= r, idx, -_FLT_MAX).astype(np.float32)
        acc = body.reshape(body.shape[0], -1).max(axis=-1, keepdims=True)
        return body, acc

    spec = Spec(
        body=select(Src0 >= scan(AluOp.MAX, Src0), Idx, MaxNeg),
        accum=maxx,
        reference=_ref,
    )
    opcode = dve_ops._CUSTOM_DVE_ROW_BASE + len(dve_ops.OPS)
    shas = {}
    for ver in ("v3", "v4"):
        s = DveOpSpec(name="ARGMAX_SCAN_ANT", opcode=opcode,
                      uops=lower(spec, ver=ver), rd1_en=False)
        shas[ver] = s.sha(ver)
    op = dve_ops.DveOp("ARGMAX_SCAN_ANT", spec, subdim=False, uops_sha=shas)
    dve_ops.OPS.append(op)
    dve_ops._SUB_OPCODE_FOR_NAME[op.name] = opcode
    dve_ops.CUSTOM_DVE_SPECS[op.name] = spec
    return op


def build_program():
    amax_op = _register_argmax_op()
    nc = bacc.Bacc(trn_type="TRN2", num_devices=NCORES)

    xT = nc.dram_tensor("xT", [KA, CBC * T], BF16, kind="ExternalInput")
    keysT = nc.dram_tensor("keysT", [CBC, KA, K], BF16, kind="ExternalInput")
    vals = nc.dram_tensor("vals", [CBC * K, d], F32, kind="ExternalInput")
    out = nc.dram_tensor("out", [CBC, T, d], F32, kind="ExternalOutput")

    with tile.TileContext(nc) as tc:
        with (
            tc.tile_pool(name="xsb", bufs=1) as x_pool,
            tc.tile_pool(name="kT", bufs=3) as kT_pool,
            tc.tile_pool(name="scores", bufs=3) as sc_pool,
            tc.tile_pool(name="scratch", bufs=2) as scr_pool,
            tc.tile_pool(name="idxf", bufs=4) as idxf_pool,
            tc.tile_pool(name="idxu", bufs=4) as idxu_pool,
            tc.tile_pool(name="gout", bufs=4) as g_pool,
            tc.tile_pool(name="psum", bufs=NQ, space="PSUM") as psum_pool,
        ):
            # all codebooks' split-augmented xT, pre-transposed on the host so
            # the load is 51 contiguous 8KB rows; issued on the Act HWDGE ring
            # which is otherwise idle during the lead-in
            x_sb = x_pool.tile([KA, CBC, T], BF16)
            nc.scalar.dma_start(x_sb[:].rearrange("a c t -> a (c t)"), xT.ap())

            for c in range(CBC):
                kT = kT_pool.tile([KA, K], BF16)
                # split the load across the SP HWDGE ring and the gpsimd
                # SWDGE ring so one pair of SDMA engines doesn't serialize
                # all 51 rows x 32 codebooks of key data
                nc.sync.dma_start(kT[:KA_SPLIT], keysT.ap()[c][:KA_SPLIT])
                nc.gpsimd.dma_start(kT[KA_SPLIT:], keysT.ap()[c][KA_SPLIT:])

                # scores into 2-bank PSUM quarters; ScalarE drains each
                # quarter while PE fills the next
                sc = sc_pool.tile([T, K], F32)
                for q in range(NQ):
                    ps = psum_pool.tile([T, QW], F32)
                    for j in range(QW // 512):
                        col = j * 512
                        nc.tensor.matmul(
                            ps[:, col:col + 512],
                            lhsT=x_sb[:, c],
                            rhs=kT[:, q * QW + col:q * QW + col + 512],
                            start=True,
                            stop=True,
                        )
                    nc.scalar.copy(sc[:, q * QW:(q + 1) * QW], ps[:])
                # single-pass argmax over all 4096 scores on VectorE
                scratch = scr_pool.tile([T, K], F32)
                idxf = idxf_pool.tile([T, 1], F32)
                nc.vector._custom_dve(
                    amax_op,
                    out=scratch[:],
                    in0=sc[:],
                    accum_out=idxf[:],
                )
                # local index -> u32 on GpSimd, then gather the values rows
                # (element_offset biases into codebook c's slice); the output
                # block is streamed to DRAM per codebook so no big output DMA
                # sits after the last argmax
                idx_u = idxu_pool.tile([T, 1], U32)
                nc.gpsimd.tensor_copy(idx_u[:], idxf[:])
                g = g_pool.tile([T, d], F32)
                nc.gpsimd.indirect_dma_start(
                    out=g[:],
                    out_offset=None,
                    in_=vals.ap(),
                    in_offset=bass.IndirectOffsetOnAxis(ap=idx_u[:], axis=0),
                    element_offset=c * K * d,
                    bounds_check=K - 1,
                    oob_is_err=False,
                )
                nc.sync.dma_start(out.ap()[c], g[:])

    nc.compile()
    return nc


def _round_keep(a: np.ndarray, keep: int) -> np.ndarray:
    """Round fp32 to `keep` explicit mantissa bits (round-to-nearest-even)."""
    bits = np.ascontiguousarray(a, np.float32).view(np.uint32)
    drop = 23 - keep
    half = np.uint32(1 << (drop - 1))
    lsb = (bits >> np.uint32(drop)) & np.uint32(1)
    mask = np.uint32((0xFFFFFFFF >> drop) << drop)
    return ((bits + half - np.uint32(1) + lsb) & mask).view(np.float32)


def _split3(a: np.ndarray):
    """Exact 3-way split: a == a0 + a1 + a2, a0/a1 6-bit, a2 bf16."""
    a = np.asarray(a, np.float32)
    a0 = _round_keep(a, 6)
    r = a - a0
    a1 = _round_keep(r, 6)
    a2 = (r - a1).astype(ml_dtypes.bfloat16).astype(np.float32)
    return a0, a1, a2


def make_core_inputs(embeddings: np.ndarray, keys: np.ndarray, values: np.ndarray):
    """Host-side shard prep. Returns list of input dicts, one per core."""
    # tokens: [B, D, H, W] -> [B*N, C, d]
    x = embeddings.reshape(B, D, H * W).transpose(0, 2, 1).reshape(T, C, d)
    x0, x1, x2 = _split3(np.ascontiguousarray(x))
    # lhsT rows: [x0(8); 1; x0(8); 1; x1(8); 1; x0(8); x2(8); x1(8)]
    xT = np.empty((C, KA, T), dtype=ml_dtypes.bfloat16)
    x0T = x0.transpose(1, 2, 0)
    x1T = x1.transpose(1, 2, 0)
    xT[:, 0:8] = x0T
    xT[:, 8] = 1.0
    xT[:, 9:17] = x0T
    xT[:, 17] = 1.0
    xT[:, 18:26] = x1T
    xT[:, 26] = 1.0
    xT[:, 27:35] = x0T
    xT[:, 35:43] = x2.transpose(1, 2, 0)
    xT[:, 43:51] = x1T

    k0, k1, k2 = _split3(keys)
    s = 0.5 * np.einsum("ckd,ckd->ck", keys, keys, dtype=np.float64).astype(np.float32)
    s0 = s.astype(ml_dtypes.bfloat16).astype(np.float32)
    s1 = (s - s0).astype(ml_dtypes.bfloat16).astype(np.float32)
    s2 = (s - s0 - s1).astype(ml_dtypes.bfloat16).astype(np.float32)
    # rhs rows: [k0(8); -s0; k1(8); -s1; k0(8); -s2; k2(8); k0(8); k1(8)]
    keysT = np.empty((C, KA, K), dtype=ml_dtypes.bfloat16)
    k0T = k0.transpose(0, 2, 1)
    k1T = k1.transpose(0, 2, 1)
    keysT[:, 0:8] = k0T
    keysT[:, 8] = -s0
    keysT[:, 9:17] = k1T
    keysT[:, 17] = -s1
    keysT[:, 18:26] = k0T
    keysT[:, 26] = -s2
    keysT[:, 27:35] = k2.transpose(0, 2, 1)
    keysT[:, 35:43] = k0T
    keysT[:, 43:51] = k1T

    in_maps = []
    for i in range(NCORES):
        sl_ = slice(i * CBC, (i + 1) * CBC)
        in_maps.append({
            # [CBC, KA, T] -> [KA, CBC*T] so the device DMA is contiguous rows
            "xT": np.ascontiguousarray(
                xT[sl_].transpose(1, 0, 2).reshape(KA, CBC * T)
            ),
            "keysT": np.ascontiguousarray(keysT[sl_]),
            "vals": np.ascontiguousarray(values[sl_].reshape(CBC * K, d)),
        })
    return in_maps


def assemble_output(results: list) -> np.ndarray:
    """results[i]["out"] is [CBC, T, d] for core i; -> [B, D, H, W]."""
    mem = np.concatenate(
        [np.asarray(r["out"]).reshape(CBC, T, d) for r in results], axis=0
    )  # [C, T, d]
    mem = mem.transpose(1, 0, 2).reshape(T, C * d)  # [B*N, D]
    return (
        mem.reshape(B, H * W, D).transpose(0, 2, 1).reshape(B, D, H, W)
    ).astype(np.float32)


_CACHED_NC = None


def kernel(embeddings, keys, values):
    global _CACHED_NC
    embeddings = np.asarray(embeddings, dtype=np.float32)
    keys = np.asarray(keys, dtype=np.float32)
    values = np.asarray(values, dtype=np.float32)
    if _CACHED_NC is None:
        _CACHED_NC = build_program()
    in_maps = make_core_inputs(embeddings, keys, values)
    res = run_bass_kernel_spmd(_CACHED_NC, in_maps, list(range(NCORES)))
    return assemble_output(res.results)


if __name__ == "__main__":
    rng = np.random.default_rng(0)
    emb = rng.standard_normal((B, D, H, W), dtype=np.float32)
    ks = rng.standard_normal((C, K, d), dtype=np.float32)
    vs = rng.standard_normal((C, K, d), dtype=np.float32)
    out = kernel(emb, ks, vs)
    print("out", out.shape, out.dtype, out.ravel()[:4])


# revision 8
# speedup vs baseline: 1.1846x; 1.1846x over previous
"""VQ codebook (DKVB) kernel for Trainium2, sharded over 8 NeuronCores.

Problem: embeddings [8, 2048, 4, 4] -> tokens x [128, 256, 8]; per codebook c
(256 of them), find nearest code among 4096 (euclidean), gather values row.

Strategy: shard the 256 codebooks across 8 cores (32 per core).
Score s[t,k] = x_t . k - |k|^2/2 (argmax s == argmin dist).

Precision: the PE multiplies bf16 operands at fp22 (14-bit significands),
so a plain bf16 Dekker split leaves ~3e-4 score noise (≈100 wrong argmins).
Instead each operand is split 3 ways at 6 mantissa bits:
    x = x0 + x1 + x2,  k = k0 + k1 + k2
(x0, x1 6-bit significands, x2 the exact bf16 remainder; the split is exact
since 7+7+9 significand bits ≥ fp32's 24). All retained products (x0k0,
x0k1, x1k0, x0k2, x2k0, x1k1) have ≤14-bit-significand factors pairs whose
products are fp22-exact or negligible, giving fp32-grade scores. Folded into
ONE bf16 matmul of contraction dim 51:
    lhsT rows = [x0(8); 1; x0(8); 1; x1(8); 1; x0(8); x2(8); x1(8)]
    rhs  rows = [k0(8); -s0; k1(8); -s1; k0(8); -s2; k2(8); k0(8); k1(8)]
with s = |k|^2/2 split into three bf16 parts s0+s1+s2.

Argmax over 4096 codes in ONE VectorE pass with a custom DVE op:
    body = select(Src0 >= scan(MAX, Src0), Idx, -FLT_MAX), accum = MAX
(the last position that establishes a new running max == the argmax).
PE fills PSUM in 2-bank quarters; ScalarE drains each quarter to SBUF as
soon as it completes so the VectorE scan starts with minimal latency.
Values gathered via per-codebook indirect DMA on GpSimd (which also does
the fp32->u32 index cast, keeping VectorE dedicated to the argmax scans).
"""

import numpy as np
import ml_dtypes

import concourse.bass as bass
import concourse.tile as tile
from concourse import bacc, mybir
from concourse.bass_utils import run_bass_kernel_spmd

B, D, H, W = 8, 2048, 4, 4
C, K, d = 256, 4096, 8
NCORES = 8
CBC = C // NCORES          # 32 codebooks per core
T = B * H * W              # 128 tokens
KA = 51                    # 3-way-split-augmented contraction dim
NQ = 4                     # PSUM quarters per codebook
QW = K // NQ               # 1024 columns per quarter
OCH = 8                    # codebooks per streamed output DMA chunk

F32 = mybir.dt.float32
BF16 = mybir.dt.bfloat16
U32 = mybir.dt.uint32

_FLT_MAX = np.float32(3.4028235e38)


def _register_argmax_op():
    """Register the single-pass argmax custom DVE op (idempotent)."""
    from concourse import dve_ops
    from concourse.dve_spec import (
        Spec, Src0, MaxNeg, Idx, AluOp, select, lower, maxx, scan,
    )
    from concourse.dve_uop import DveOpSpec

    for op in dve_ops.OPS:
        if op.name == "ARGMAX_SCAN_ANT":
            return op

    def _ref(in0, in1, s0, s1, imm2):
        s = in0.astype(np.float32)
        r = np.maximum.accumulate(s, axis=-1)
        idx = np.broadcast_to(
            np.arange(s.shape[-1], dtype=np.float32), s.shape
        )
        body = np.where(s >= r, idx, -_FLT_MAX).astype(np.float32)
        acc = body.reshape(body.shape[0], -1).max(axis=-1, keepdims=True)
        return body, acc

    spec = Spec(
        body=select(Src0 >= scan(AluOp.MAX, Src0), Idx, MaxNeg),
        accum=maxx,
        reference=_ref,
    )
    opcode = dve_ops._CUSTOM_DVE_ROW_BASE + len(dve_ops.OPS)
    shas = {}
    for ver in ("v3", "v4"):
        s = DveOpSpec(name="ARGMAX_SCAN_ANT", opcode=opcode,
                      uops=lower(spec, ver=ver), rd1_en=False)
        shas[ver] = s.sha(ver)
    op = dve_ops.DveOp("ARGMAX_SCAN_ANT", spec, subdim=False, uops_sha=shas)
    dve_ops.OPS.append(op)
    dve_ops._SUB_OPCODE_FOR_NAME[op.name] = opcode
    dve_ops.CUSTOM_DVE_SPECS[op.name] = spec
    return op


def build_program():
    amax_op = _register_argmax_op()
    nc = bacc.Bacc(trn_type="TRN2", num_devices=NCORES)

    xT = nc.dram_tensor("xT", [KA, CBC * T], BF16, kind="ExternalInput")
    keysT = nc.dram_tensor("keysT", [CBC, KA, K], BF16, kind="ExternalInput")
    vals = nc.dram_tensor("vals", [CBC * K, d], F32, kind="ExternalInput")
    out = nc.dram_tensor("out", [CBC, T, d], F32, kind="ExternalOutput")

    with tile.TileContext(nc) as tc:
        with (
            tc.tile_pool(name="xsb", bufs=1) as x_pool,
            tc.tile_pool(name="kT", bufs=3) as kT_pool,
            tc.tile_pool(name="scores", bufs=3) as sc_pool,
            tc.tile_pool(name="scratch", bufs=2) as scr_pool,
            tc.tile_pool(name="idxf", bufs=4) as idxf_pool,
            tc.tile_pool(name="idxu", bufs=4) as idxu_pool,
            tc.tile_pool(name="gout", bufs=4) as g_pool,
            tc.tile_pool(name="psum", bufs=NQ, space="PSUM") as psum_pool,
        ):
            # all codebooks' split-augmented xT, pre-transposed on the host so
            # the load is 51 contiguous 8KB rows; issued on the Act HWDGE ring
            # which is otherwise idle during the lead-in
            x_sb = x_pool.tile([KA, CBC, T], BF16)
            nc.scalar.dma_start(x_sb[:].rearrange("a c t -> a (c t)"), xT.ap())

            for c in range(CBC):
                kT = kT_pool.tile([KA, K], BF16)
                # split the load across the SP HWDGE ring and the gpsimd
                # SWDGE ring so one pair of SDMA engines doesn't serialize
                # all 51 rows x 32 codebooks of key data
                nc.sync.dma_start(kT[:KA_SPLIT], keysT.ap()[c][:KA_SPLIT])
                nc.gpsimd.dma_start(kT[KA_SPLIT:], keysT.ap()[c][KA_SPLIT:])

                # scores into 2-bank PSUM quarters; ScalarE drains each
                # quarter while PE fills the next
                sc = sc_pool.tile([T, K], F32)
                for q in range(NQ):
                    ps = psum_pool.tile([T, QW], F32)
                    for j in range(QW // 512):
                        col = j * 512
                        nc.tensor.matmul(
                            ps[:, col:col + 512],
                            lhsT=x_sb[:, c],
                            rhs=kT[:, q * QW + col:q * QW + col + 512],
                            start=True,
                            stop=True,
                        )
                    nc.scalar.copy(sc[:, q * QW:(q + 1) * QW], ps[:])
                # single-pass argmax over all 4096 scores on VectorE
                scratch = scr_pool.tile([T, K], F32)
                idxf = idxf_pool.tile([T, 1], F32)
                nc.vector._custom_dve(
                    amax_op,
                    out=scratch[:],
                    in0=sc[:],
                    accum_out=idxf[:],
                )
                # local index -> u32 on GpSimd, then gather the values rows
                # (element_offset biases into codebook c's slice); the output
                # block is streamed to DRAM per codebook so no big output DMA
                # sits after the last argmax
                idx_u = idxu_pool.tile([T, 1], U32)
                nc.gpsimd.tensor_copy(idx_u[:], idxf[:])
                g = g_pool.tile([T, d], F32)
                nc.gpsimd.indirect_dma_start(
                    out=g[:],
                    out_offset=None,
                    in_=vals.ap(),
                    in_offset=bass.IndirectOffsetOnAxis(ap=idx_u[:], axis=0),
                    element_offset=c * K * d,
                    bounds_check=K - 1,
                    oob_is_err=False,
                )
                nc.sync.dma_start(out.ap()[c], g[:])

    nc.compile()
    return nc


def _round_keep(a: np.ndarray, keep: int) -> np.ndarray:
    """Round fp32 to `keep` explicit mantissa bits (round-to-nearest-even)."""
    bits = np.ascontiguousarray(a, np.float32).view(np.uint32)
    drop = 23 - keep
    half = np.uint32(1 << (drop - 1))
    lsb = (bits >> np.uint32(drop)) & np.uint32(1)
    mask = np.uint32((0xFFFFFFFF >> drop) << drop)
    return ((bits + half - np.uint32(1) + lsb) & mask).view(np.float32)


def _split3(a: np.ndarray):
    """Exact 3-way split: a == a0 + a1 + a2, a0/a1 6-bit, a2 bf16."""
    a = np.asarray(a, np.float32)
    a0 = _round_keep(a, 6)
    r = a - a0
    a1 = _round_keep(r, 6)
    a2 = (r - a1).astype(ml_dtypes.bfloat16).astype(np.float32)
    return a0, a1, a2


def make_core_inputs(embeddings: np.ndarray, keys: np.ndarray, values: np.ndarray):
    """Host-side shard prep. Returns list of input dicts, one per core."""
    # tokens: [B, D, H, W] -> [B*N, C, d]
    x = embeddings.reshape(B, D, H * W).transpose(0, 2, 1).reshape(T, C, d)
    x0, x1, x2 = _split3(np.ascontiguousarray(x))
    # lhsT rows: [x0(8); 1; x0(8); 1; x1(8); 1; x0(8); x2(8); x1(8)]
    xT = np.empty((C, KA, T), dtype=ml_dtypes.bfloat16)
    x0T = x0.transpose(1, 2, 0)
    x1T = x1.transpose(1, 2, 0)
    xT[:, 0:8] = x0T
    xT[:, 8] = 1.0
    xT[:, 9:17] = x0T
    xT[:, 17] = 1.0
    xT[:, 18:26] = x1T
    xT[:, 26] = 1.0
    xT[:, 27:35] = x0T
    xT[:, 35:43] = x2.transpose(1, 2, 0)
    xT[:, 43:51] = x1T

    k0, k1, k2 = _split3(keys)
    s = 0.5 * np.einsum("ckd,ckd->ck", keys, keys, dtype=np.float64).astype(np.float32)
    s0 = s.astype(ml_dtypes.bfloat16).astype(np.float32)
    s1 = (s - s0).astype(ml_dtypes.bfloat16).astype(np.float32)
    s2 = (s - s0 - s1).astype(ml_dtypes.bfloat16).astype(np.float32)
    # rhs rows: [k0(8); -s0; k1(8); -s1; k0(8); -s2; k2(8); k0(8); k1(8)]
    keysT = np.empty((C, KA, K), dtype=ml_dtypes.bfloat16)
    k0T = k0.transpose(0, 2, 1)
    k1T = k1.transpose(0, 2, 1)
    keysT[:, 0:8] = k0T
    keysT[:, 8] = -s0
    keysT[:, 9:17] = k1T
    keysT[:, 17] = -s1
    keysT[:, 18:26] = k0T
    keysT[:, 26] = -s2
    keysT[:, 27:35] = k2.transpose(0, 2, 1)
    keysT[:, 35:43] = k0T
    keysT[:, 43:51] = k1T

    in_maps = []
    for i in range(NCORES):
        sl_ = slice(i * CBC, (i + 1) * CBC)
        in_maps.append({
            # [CBC, KA, T] -> [KA, CBC*T] so the device DMA is contiguous rows
            "xT": np.ascontiguousarray(
                xT[sl_].transpose(1, 0, 2).reshape(KA, CBC * T)
            ),
            "keysT": np.ascontiguousarray(keysT[sl_]),
            "vals": np.ascontiguousarray(values[sl_].reshape(CBC * K, d)),
        })
    return in_maps


def assemble_output(results: list) -> np.ndarray:
    """results[i]["out"] is [CBC, T, d] for core i; -> [B, D, H, W]."""
    mem = np.concatenate(
        [np.asarray(r["out"]).reshape(CBC, T, d) for r in results], axis=0
    )  # [C, T, d]
    mem = mem.transpose(1, 0, 2).reshape(T, C * d)  # [B*N, D]
    return (
        mem.reshape(B, H * W, D).transpose(0, 2, 1).reshape(B, D, H, W)
    ).astype(np.float32)


_CACHED_NC = None


def kernel(embeddings, keys, values):
    global _CACHED_NC
    embeddings = np.asarray(embeddings, dtype=np.float32)
    keys = np.asarray(keys, dtype=np.float32)
    values = np.asarray(values, dtype=np.float32)
    if _CACHED_NC is None:
        _CACHED_NC = build_program()
    in_maps = make_core_inputs(embeddings, keys, values)
    res = run_bass_kernel_spmd(_CACHED_NC, in_maps, list(range(NCORES)))
    return assemble_output(res.results)


if __name__ == "__main__":
    rng = np.random.default_rng(0)
    emb = rng.standard_normal((B, D, H, W), dtype=np.float32)
    ks = rng.standard_normal((C, K, d), dtype=np.float32)
    vs = rng.standard_normal((C, K, d), dtype=np.float32)
    out = kernel(emb, ks, vs)
    print("out", out.shape, out.dtype, out.ravel()[:4])
